# revision 18
# baseline (speedup 1.0000x reference)
"""DeepseekV2 MoE layer on 8 Trainium2 NeuronCores (Bass/Tile, SPMD).

Strategy (expert-parallel with intermediate-dim pair-split, bf16 matmuls):
 - Host computes the MoE gate routing in numpy (bitwise-matches the jax
   reference: top-k margins are ~1e-4, far above ulp noise).
 - 16 experts, rank-sorted by token count, are dealt into 4 groups of 4
   (group p = ranks {p, 4+p, 8+p, 12+p}).  Group p is owned by the core
   pair (2p, 2p+1): both cores process the SAME gathered tokens of all 4
   experts, but each core only computes HALF of every expert's
   intermediate dim (704 rows, zero-padded to 768 = 6 tile-pairs).  The
   two half outputs are partial sums; the host adds them.  This beats
   whole-expert placement because per-slot capacity is the max count at
   ranks {0,4,8,12} (688+432+352+272) instead of ranks {0,8} (688+352)
   at double width.
 - Device per core: for each of 4 slots, GEMM1 (x_gathered^T @ half
   w_gate_up^T, gate/up rows interleaved in 128-row pairs) -> SiLU*mul
   -> GEMM2 emitted TOKEN-MAJOR: psum partitions are D-channels (16
   fixed stripes) and the gathered tokens ride the moving dim, so slot
   capacities need no 128-row quantization.  Outputs leave as y^T [D,
   cap] in bf16; the host applies the per-token combine weight (gate
   weight * 2.5) during its scatter-add.  Then the shared expert,
   TP-sharded over its intermediate dim (352/core, zero-padded to 384).
 - Phases are emitted interleaved (GEMM1 of slot s+1 before GEMM2 of
   slot s) so the PE never idles on a slot's silu/mul tail.
 - All matmuls run in bf16 (fp32 PSUM accumulate): halves HBM traffic
   and enables fast-weight-load; rel err ~5e-3 vs the 2e-2 gate.
 - Host scatter-adds per-expert partial outputs and shared partials, f32.
"""

import numpy as np
import ml_dtypes
from contextlib import ExitStack

import concourse.bacc as bacc
import concourse.tile as tile
import concourse.mybir as mybir
from concourse.bass_utils import run_bass_kernel_spmd

# problem dims (fixed by the graded problem)
T, D, I, E = 1024, 2048, 1408, 16
SI = 2 * I               # shared expert intermediate (2816)
TOP_K, N_GROUP, TOPK_GROUP = 6, 4, 2
ROUTED_SCALE = 2.5
NCORES = 8
KT = D // 128            # 16 contraction tiles
IH = I // 2              # 704 intermediate rows per half
IHP = 768                # padded to 6x128
HIT = IHP // 128         # 6 intermediate tiles per half
SSLICE = SI // NCORES    # 352 shared-intermediate rows per core
SIP = 384                # padded to 3x128
SIT = SIP // 128         # 3
DS = D // 128            # 16 gemm2 output stripes (D-channels)
NSLOT = 4                # expert slots per core

f32 = mybir.dt.float32
bf16 = mybir.dt.bfloat16
np_bf16 = ml_dtypes.bfloat16
ACT_SILU = mybir.ActivationFunctionType.Silu
ACT_SIGMOID = mybir.ActivationFunctionType.Sigmoid
_SIM_SILU = False  # CoreSim lacks Silu; True emits Sigmoid + explicit mul


# ---------------------------------------------------------------- routing
def _route(x, gate_w, bias):
    """Replicates the jax reference gate in numpy f32 (decision margins are
    >=1e-4 so ulp-level differences cannot flip the top-k).

    Returns topk_idx [T,6] int, weights [T,6] f32 (renormalized, unscaled).
    """
    logits = (x @ gate_w.T).astype(np.float32)
    scores = (1.0 / (1.0 + np.exp(-logits))).astype(np.float32)
    s_choice = scores + bias.astype(np.float32)
    grp = s_choice.reshape(T, N_GROUP, E // N_GROUP)
    group_scores = np.sort(grp, axis=2)[:, :, -2:].sum(2, dtype=np.float32)
    grp_idx = np.argsort(-group_scores, axis=1, kind="stable")[:, :TOPK_GROUP]
    gmask = np.zeros((T, N_GROUP), dtype=bool)
    gmask[np.arange(T)[:, None], grp_idx] = True
    emask = np.repeat(gmask, E // N_GROUP, axis=1)
    masked = np.where(emask, s_choice, -np.inf)
    topk_idx = np.argsort(-masked, axis=1, kind="stable")[:, :TOP_K]
    w = np.take_along_axis(scores, topk_idx, axis=1)
    w = (w / w.sum(axis=1, keepdims=True)).astype(np.float32)
    return topk_idx, w


def _chunks(c):
    """Split capacity c into GEMM moving-dim chunks, each <= 512."""
    if c <= 512:
        return [(0, c)]
    a = 16 * ((c + 31) // 32)
    return [(0, a), (a, c - a)]


def _pad16(n):
    return max(128, 16 * ((n + 15) // 16))


# ------------------------------------------------------------ host packing
def _pack_wgu(w, it_cnt):
    """w: [2*ic, D] rows (gate block then up block, ic=128*it_cnt rows each)
    -> [2*it_cnt, 128, KT, 128] bf16 with gate/up 128-row tiles interleaved,
    laid out so tile m is a [128 part, KT*128] contiguous block of
    w^T[k-tile, m-tile]."""
    ic = 128 * it_cnt
    g = w[:ic].reshape(it_cnt, 128, D)
    u = w[ic:].reshape(it_cnt, 128, D)
    inter = np.stack([g, u], axis=1).reshape(2 * it_cnt * 128, D)  # [2ic, D]
    t = inter.T.reshape(KT, 128, 2 * it_cnt, 128).transpose(2, 1, 0, 3)
    return np.ascontiguousarray(t).astype(np_bf16)


def _pack_wdT(wdT, it_cnt):
    """wdT: [128*it_cnt, D] (= w_down^T, zero-padded rows allowed)
    -> [128, it_cnt, D] bf16 (partition = intermediate channel within its
    k-tile), for token-major GEMM2 (stationary [128, 128-D-stripe] slices)."""
    t = wdT.reshape(it_cnt, 128, D).transpose(1, 0, 2)
    return np.ascontiguousarray(t).astype(np_bf16)


def _pack_xT(xs, cap):
    """xs: [n, D] token rows -> [128, KT, cap] bf16 (x^T k-tiles, padded)."""
    out = np.zeros((128, KT, cap), dtype=np_bf16)
    n = xs.shape[0]
    out[:, :, :n] = xs.T.reshape(KT, 128, n).transpose(1, 0, 2).astype(np_bf16)
    return out


# ------------------------------------------------------------ device build
def _build(caps):
    nc = bacc.Bacc("TRN2", target_bir_lowering=False, debug=False,
                   num_devices=NCORES)

    xg_d = [nc.dram_tensor(f"xg{s}", [128, KT, caps[s]], bf16,
                           kind="ExternalInput") for s in range(NSLOT)]
    wgu_d = [nc.dram_tensor(f"wgu{s}", [2 * HIT, 128, KT, 128], bf16,
                            kind="ExternalInput") for s in range(NSLOT)]
    wd_d = [nc.dram_tensor(f"wd{s}", [128, HIT, D], bf16,
                           kind="ExternalInput") for s in range(NSLOT)]
    yr_d = [nc.dram_tensor(f"yr{s}", [D, caps[s]], bf16,
                           kind="ExternalOutput") for s in range(NSLOT)]
    xt_d = nc.dram_tensor("xt", [128, KT, T], bf16, kind="ExternalInput")
    wsgu_d = nc.dram_tensor("wsgu", [2 * SIT, 128, KT, 128], bf16, kind="ExternalInput")
    wsd_d = nc.dram_tensor("wsd", [128, SIT, D], bf16, kind="ExternalInput")
    ys_d = nc.dram_tensor("ys", [D, T], bf16, kind="ExternalOutput")

    with tile.TileContext(nc) as tc, ExitStack() as ctx:
        sb = ctx.enter_context(tc.tile_pool(name="sb", bufs=1))
        ps = ctx.enter_context(tc.tile_pool(name="ps", bufs=1, space="PSUM"))

        def gemm1(xgd, cap, chunks, it_cnt, wgud, wdd, xtag, kblk=4):
            """Emit x load + wdf prefetch + GEMM1 + silu*mul.

            The slot's down-projection weights (wdf) are DMA'd here — ahead
            of the next slot's wgu stream in the sync FIFO — so GEMM2 never
            waits on them at the phase boundary.  Returns (at, wdf)."""
            # per-phase x tag (bufs=1, exclusive) so every phase's x load can
            # start at kernel t=0; k-block sub-DMAs let matmuls chase the
            # transfer at k-granularity (finer for the first phase).
            xg = sb.tile([128, KT, cap], bf16, tag=xtag, bufs=1, name=xtag)
            for kb in range(0, KT, kblk):
                nc.gpsimd.dma_start(xg[:, kb:kb + kblk, :], xgd.ap()[:, kb:kb + kblk, :])
            xg_at = lambda k: xg[:, k, :]
            # wdf goes via gpsimd (SWDGE) so it never head-of-line blocks the
            # wgu stream on the sync HWDGE FIFO.
            wdf = sb.tile([128, it_cnt, D], bf16, tag="wdf", bufs=2, name="wdf")
            for q in range(4):
                nc.gpsimd.dma_start(wdf[:, :, q * 512:(q + 1) * 512],
                                    wdd.ap()[:, :, q * 512:(q + 1) * 512])
            at = sb.tile([128, it_cnt, cap], bf16, tag="at", bufs=3, name="at")
            for t in range(it_cnt):
                pair = []
                for par in (0, 1):
                    wgu = sb.tile([128, KT, 128], bf16, tag="wgu", bufs=7, name="wgu")
                    h8 = KT // 2
                    nc.sync.dma_start(wgu[:, :h8, :], wgud.ap()[2 * t + par, :, :h8, :])
                    nc.sync.dma_start(wgu[:, h8:, :], wgud.ap()[2 * t + par, :, h8:, :])
                    row = []
                    for off, n in chunks:
                        p = ps.tile([128, n], f32, tag=f"ps{par}", bufs=3, name=f"ps{par}")
                        for k in range(KT):
                            nc.tensor.matmul(p[:], wgu[:, k, :], xg_at(k)[:, off:off + n],
                                             start=(k == 0), stop=(k == KT - 1))
                        row.append(p)
                    pair.append(row)
                for ci, (off, n) in enumerate(chunks):
                    tmp = sb.tile([128, n], bf16, tag="tmp", bufs=2, name="tmp")
                    if _SIM_SILU:
                        nc.scalar.activation(tmp[:], pair[0][ci][:], ACT_SIGMOID)
                        nc.vector.tensor_mul(tmp[:], tmp[:], pair[0][ci][:])
                    else:
                        nc.scalar.activation(tmp[:], pair[0][ci][:], ACT_SILU)
                    nc.vector.tensor_mul(at[:, t, off:off + n], tmp[:], pair[1][ci][:])
            return at, wdf

        def gemm2(at, wdf, chunks, it_cnt, out_d):
            """Token-major GEMM2: psum partitions are D-channels, tokens ride
            the moving dim (no 128-row quantization of slot capacity)."""
            for ds in range(DS):
                for off, n in chunks:
                    yp = ps.tile([128, 512], f32, tag="psy", bufs=2, name="yp")
                    for k in range(it_cnt):
                        nc.tensor.matmul(yp[:, :n], wdf[:, k, ds * 128:(ds + 1) * 128],
                                         at[:, k, off:off + n],
                                         start=(k == 0), stop=(k == it_cnt - 1))
                    ysb = sb.tile([128, 512], bf16, tag="ysb", bufs=3, name="ysb")
                    nc.vector.tensor_copy(ysb[:, :n], yp[:, :n])
                    nc.scalar.dma_start(out_d.ap()[ds * 128:(ds + 1) * 128, off:off + n],
                                        ysb[:, :n])

        # slots largest-first (weight-DMA-hungry small slots get prefetch
        # slack); shared expert last.  GEMM1 of phase i+1 is emitted before
        # GEMM2 of phase i so the PE stream never waits on a silu/mul tail.
        phases = []
        for s in range(NSLOT):
            phases.append(dict(xgd=xg_d[s], cap=caps[s], chunks=_chunks(caps[s]),
                               it=HIT, wgud=wgu_d[s], wdd=wd_d[s], out=yr_d[s],
                               xtag=f"xb{s}", kblk=4))
        phases.append(dict(xgd=xt_d, cap=T, chunks=[(0, 512), (512, 512)],
                           it=SIT, wgud=wsgu_d, wdd=wsd_d, out=ys_d,
                           xtag="xb0", kblk=4))
        phases[0]["kblk"] = 2
        pend = None
        for ph in phases:
            a, wdf = gemm1(ph["xgd"], ph["cap"], ph["chunks"], ph["it"],
                           ph["wgud"], ph["wdd"], ph["xtag"], ph["kblk"])
            if pend is not None:
                gemm2(pend[0], pend[1], pend[2]["chunks"], pend[2]["it"],
                      pend[2]["out"])
            pend = (a, wdf, ph)
        gemm2(pend[0], pend[1], pend[2]["chunks"], pend[2]["it"],
              pend[2]["out"])

    nc.compile()
    return nc


# ----------------------------------------------------------------- kernel
def kernel(x, gate_w, bias, w_gate_up, w_down, shared_w_gate_up,
           shared_w_down, _trace=False):
    x = np.ascontiguousarray(x, dtype=np.float32)
    topk_idx, w = _route(x, gate_w, bias)
    cw_full = w.astype(np.float32) * np.float32(ROUTED_SCALE)

    # expert -> token list + weight list
    toks, wts, counts = [], [], np.zeros(E, dtype=np.int64)
    for e in range(E):
        tsel, ksel = np.where(topk_idx == e)
        toks.append(tsel)
        wts.append(cw_full[tsel, ksel])
        counts[e] = len(tsel)

    # rank-sorted experts dealt into 4 slots x 4 groups; group p -> cores
    # (2p, 2p+1), each core computing one half of the intermediate dim.
    order = np.argsort(-counts, kind="stable")
    slot_experts = [[int(order[4 * s + p]) for p in range(4)] for s in range(NSLOT)]
    caps = [_pad16(int(max(counts[e] for e in slot_experts[s])))
            for s in range(NSLOT)]

    # pack per (group, slot, half) once; xg shared by both cores of a pair
    xt_pack = _pack_xT(x, T)
    in_maps = []
    for c in range(NCORES):
        p, h = c // 2, c % 2
        m = {}
        for s in range(NSLOT):
            eid = slot_experts[s][p]
            if h == 0:
                m[f"xg{s}"] = _pack_xT(x[toks[eid]], caps[s])
            else:
                m[f"xg{s}"] = in_maps[c - 1][f"xg{s}"]
            # half h of the expert's intermediate rows, zero-padded 704->768
            gsl = np.zeros((2 * IHP, D), dtype=np.float32)
            gsl[:IH] = w_gate_up[eid][IH * h: IH * (h + 1)]
            gsl[IHP:IHP + IH] = w_gate_up[eid][I + IH * h: I + IH * (h + 1)]
            m[f"wgu{s}"] = _pack_wgu(gsl, HIT)
            sdT = np.zeros((IHP, D), dtype=np.float32)
            sdT[:IH] = w_down[eid].T[IH * h: IH * (h + 1)]
            m[f"wd{s}"] = _pack_wdT(sdT, HIT)
        # shared expert slice (rows [352c, 352c+352), zero-padded to 384)
        gsl = np.zeros((2 * SIP, D), dtype=np.float32)
        gsl[:SSLICE] = shared_w_gate_up[SSLICE * c: SSLICE * (c + 1)]
        gsl[SIP:SIP + SSLICE] = shared_w_gate_up[SI + SSLICE * c: SI + SSLICE * (c + 1)]
        m["wsgu"] = _pack_wgu(gsl, SIT)
        sdT = np.zeros((SIP, D), dtype=np.float32)
        sdT[:SSLICE] = shared_w_down[:, SSLICE * c: SSLICE * (c + 1)].T
        m["wsd"] = _pack_wdT(sdT, SIT)
        m["xt"] = xt_pack
        in_maps.append(m)

    nc = _build(caps)
    kw = {}
    if _trace:
        kw = dict(trace=True, trace_cores=list(range(NCORES)))
    res = run_bass_kernel_spmd(nc, in_maps, core_ids=list(range(NCORES)), **kw)

    y = np.zeros((T, D), dtype=np.float32)
    for c in range(NCORES):
        y += res.results[c]["ys"].astype(np.float32).T
    for p in range(NCORES // 2):
        for s in range(NSLOT):
            eid = slot_experts[s][p]
            n = int(counts[eid])
            if n:
                acc = (res.results[2 * p][f"yr{s}"][:, :n].astype(np.float32) +
                       res.results[2 * p + 1][f"yr{s}"][:, :n].astype(np.float32))
                y[toks[eid]] += acc.T * wts[eid][:, None]
    if _trace:
        return y, res
    return y
